# revision 1
# baseline (speedup 1.0000x reference)
"""Trainium2 Bass kernel for nn_Attention_11458972746115.

Multi-head attention (B=1, S=2048, D=1024, H=16, DH=64) with RoPE and a
block-diagonal segment mask, tensor-parallel over heads across 8 NeuronCores
(2 heads per core).  Each core computes qkv projections, RoPE, block-sparse
masked attention and its slice of the output projection; the partial output
products (sum-sharded over the wo contraction) are reduced on the host.

Key device-side tricks:
 - All big matmuls run as float32r (full-rate fp32 mode, 1 cycle/row for
   free dim >= 256; operands must be produced as f32r).
 - Scores are computed transposed (k on partitions, q on free) so softmax
   renormalization can ride the attn@v matmul: v is augmented with a ones
   column, producing the softmax denominator as an extra psum row.
 - The segment mask is folded into the score matmul itself by augmenting the
   contraction dim with 5 extra rows (+/-256 one-hot segment indicators), so
   masked entries come out at score-256 and exp() underflows them to exactly 0.
 - RoPE pair-rotation is done with two elementwise multiplies (cos/sin) plus
   a pair-swap permutation matmul on the PE, which also relocates each head's
   rows to partitions 0..63.
 - Attention is block-sparse: only (k-tile, q-chunk) pairs whose segments
   overlap are computed (segment boundaries read from seg_ids on the host).
 - Softmax 1/r is broadcast across a head's 64 partitions with a tiny
   ones-outer-product matmul and applied during the psum->sbuf drain, so no
   separate renormalization pass over the full matrix is needed.
 - PSUM is laid out so the projection/rope front (3 banks), score pipeline
   (3), and attn@v accumulators (2) coexist: attention for early segments
   overlaps the tail of the projection phase, and the output projection for
   finished 512-column prefixes overlaps later attention.  The two heads'
   score pipelines are interleaved per query chunk for scheduling slack.
"""

import os
import numpy as np

S = 2048
D = 1024
H = 16
DH = 64
NCORES = 8
NKT = S // 128  # 16 k-tiles

_PROG_CACHE = {}


def _chunks(lo, hi, maxw=512):
    n = -(-(hi - lo) // maxw)
    base = (hi - lo) // n
    rem = (hi - lo) % n
    out = []
    p = lo
    for i in range(n):
        w = base + (1 if i < rem else 0)
        out.append((p, p + w))
        p += w
    return out


def _build(bounds, reps=1):
    import contextlib

    import concourse.bacc as bacc
    import concourse.bass as bass
    import concourse.mybir as mybir
    import concourse.tile as tile
    from concourse.bass import ts

    f32 = mybir.dt.float32
    f32r = mybir.dt.float32r
    AF = mybir.ActivationFunctionType

    segs = [(bounds[g], bounds[g + 1]) for g in range(4) if bounds[g + 1] > bounds[g]]

    def seg_windows(lo, hi):
        wins = [(w0, min(w0 + 128, hi)) for w0 in range(lo, hi, 128)]
        if all(w0 <= S - 128 for (w0, _) in wins):
            return wins
        # fallback: 128-aligned tiles spanning the segment
        return [(t * 128, min((t + 1) * 128, S))
                for t in range(lo // 128, (hi - 1) // 128 + 1)]

    allwins = []
    winidx = {}
    for (lo, hi) in segs:
        lst = []
        for (w0, w1) in seg_windows(lo, hi):
            winidx[(w0, w1)] = len(allwins)
            lst.append((len(allwins), w0, w1))
            allwins.append((w0, w1))
        winidx[(lo, hi)] = lst
    NW = len(allwins)

    nc = bacc.Bacc(None, target_bir_lowering=False)

    xts_d = nc.dram_tensor("xts", [128, 8 * S], f32r, kind="ExternalInput")
    wqs_d = nc.dram_tensor("wqs", [128, 1024], f32r, kind="ExternalInput")
    wks_d = nc.dram_tensor("wks", [128, 1024], f32r, kind="ExternalInput")
    wvs_d = nc.dram_tensor("wvs", [128, 1024], f32r, kind="ExternalInput")
    wos_d = nc.dram_tensor("wos", [128, 1024], f32r, kind="ExternalInput")
    cs_d = nc.dram_tensor("cs2", [128, S], f32, kind="ExternalInput")
    sn_d = nc.dram_tensor("sn2", [128, S], f32, kind="ExternalInput")
    mq_d = nc.dram_tensor("mskq", [5, S], f32r, kind="ExternalInput")
    mk_d = nc.dram_tensor("mskk", [5, S], f32r, kind="ExternalInput")
    sel_d = nc.dram_tensor("sels", [128, 256], f32r, kind="ExternalInput")
    io_d = nc.dram_tensor("idon", [128, 192], f32r, kind="ExternalInput")
    out_d = nc.dram_tensor("outp", [S, D], f32, kind="ExternalOutput")

    with tile.TileContext(nc) as tc:
        rep_ctx = (tc.For_i(0, reps, 1, hint_engines=(
            mybir.EngineType.PE, mybir.EngineType.DVE,
            mybir.EngineType.Activation, mybir.EngineType.SP,
            mybir.EngineType.Pool))
                   if reps > 1 else contextlib.nullcontext())
        with rep_ctx, \
             tc.tile_pool(name="fp", bufs=1, space="PSUM") as FP, \
             tc.tile_pool(name="apsum", bufs=3, space="PSUM") as AP_, \
             tc.tile_pool(name="bp", bufs=2, space="PSUM") as BP, \
             tc.tile_pool(name="const", bufs=1) as const, \
             tc.tile_pool(name="pers", bufs=1) as pers, \
             tc.tile_pool(name="xp", bufs=1) as xpool, \
             tc.tile_pool(name="abp", bufs=1) as abp, \
             tc.tile_pool(name="up", bufs=3) as upool, \
             tc.tile_pool(name="ocp", bufs=3) as ocp:

            wq_sb = const.tile([128, 1024], f32r)
            wk_sb = const.tile([128, 1024], f32r)
            wv_sb = const.tile([128, 1024], f32r)
            wo0_sb = const.tile([64, 1024], f32r)
            wo1_sb = const.tile([64, 1024], f32r)
            cs_sb = const.tile([128, S], f32)
            sn_sb = const.tile([128, S], f32)
            sel_sb = const.tile([128, 256], f32r)
            io_sb = const.tile([128, 192], f32r)
            for w_sb, w_d in ((wq_sb, wqs_d), (wk_sb, wks_d), (wv_sb, wvs_d)):
                nc.sync.dma_start(w_sb[:], w_d[:])
            nc.sync.dma_start(cs_sb[:], cs_d[:])
            nc.sync.dma_start(sn_sb[:], sn_d[:])
            id_sb = io_sb[:, 0:128]
            ones_sb = io_sb[:, 128:192]

            qa = [pers.tile([69, S], f32r, name=f"qa{h}") for h in (0, 1)]
            ka = [pers.tile([69, S], f32r, name=f"ka{h}") for h in (0, 1)]
            vga = pers.tile([128, NW, 130], f32r, name="vga")
            vg = [vga[:, :, 0:65], vga[:, :, 65:130]]
            vt_sb = pers.tile([128, S], f32r)
            osb = [pers.tile([64, S], f32r, name=f"o{h}") for h in (0, 1)]
            rsb = [pers.tile([65, S], f32r, name=f"r{h}") for h in (0, 1)]

            ab = {}
            ngrp = [0]
            npout = [0]

            def front_half(half):
                xt = []
                for d in range(8):
                    xti = xpool.tile([128, 1024], f32r, tag=f"xt{d}",
                                     name=f"xt{half}_{d}")
                    c0 = d * S + half * 1024
                    nc.sync.dma_start(xti[:], xts_d[:, c0:c0 + 1024])
                    xt.append(xti)
                if half == 0:
                    nc.sync.dma_start(sel_sb[:], sel_d[:])
                    nc.sync.dma_start(io_sb[:], io_d[:])
                    for h in (0, 1):
                        nc.sync.dma_start(qa[h][64:69, :], mq_d[:])
                        nc.sync.dma_start(ka[h][64:69, :], mk_d[:])
                    ones_cols = vga.rearrange(
                        "p w (a c) -> p w a c", a=2, c=65)[:, :, :, 64:65]
                    nc.gpsimd.memset(ones_cols.bitcast(mybir.dt.uint32),
                                     0x3F800000)
                else:
                    nc.sync.dma_start(wo0_sb[:], wos_d[0:64, :])
                    nc.sync.dma_start(wo1_sb[:], wos_d[64:128, :])
                aq = abp.tile([128, 1024], f32r, tag="aq", name=f"aq{half}")
                bq = abp.tile([128, 1024], f32r, tag="bq", name=f"bq{half}")
                ak = abp.tile([128, 1024], f32r, tag="ak", name=f"ak{half}")
                bk = abp.tile([128, 1024], f32r, tag="bk", name=f"bk{half}")
                ab.update(aq=aq, bq=bq, ak=ak, bk=bk)
                for c in (0, 1):
                    c0 = half * 1024 + c * 512
                    for proj, w_sb in (("q", wq_sb), ("k", wk_sb), ("v", wv_sb)):
                        bank = FP.tile([128, 512], f32,
                                       tag=f"pqk{ngrp[0] % 2}",
                                       name=f"p{proj}{half}{c}")
                        ngrp[0] += 1
                        for d in range(8):
                            nc.tensor.matmul(bank[:], w_sb[:, ts(d, 128)],
                                             xt[d][:, ts(c, 512)],
                                             start=(d == 0), stop=(d == 7))
                        if proj == "q":
                            nc.vector.tensor_mul(aq[:, ts(c, 512)], bank[:],
                                                 cs_sb[:, c0:c0 + 512])
                            nc.vector.tensor_mul(bq[:, ts(c, 512)], bank[:],
                                                 sn_sb[:, c0:c0 + 512])
                        elif proj == "k":
                            nc.vector.tensor_mul(ak[:, ts(c, 512)], bank[:],
                                                 cs_sb[:, c0:c0 + 512])
                            nc.vector.tensor_mul(bk[:, ts(c, 512)], bank[:],
                                                 sn_sb[:, c0:c0 + 512])
                        else:
                            nc.scalar.copy(vt_sb[:, c0:c0 + 512], bank[:])
                    for (a_t, b_t, dst) in ((aq, bq, qa), (ak, bk, ka)):
                        for h in (0, 1):
                            psel = sel_sb[:, ts(h, 64)]
                            isel = sel_sb[:, 128 + h * 64: 128 + h * 64 + 64]
                            rp = FP.tile([64, 512], f32, tag="rp", name="rp")
                            nc.tensor.matmul(rp[:], psel, b_t[:, ts(c, 512)],
                                             start=True, stop=False)
                            nc.tensor.matmul(rp[:], isel, a_t[:, ts(c, 512)],
                                             start=False, stop=True)
                            nc.scalar.copy(dst[h][0:64, c0:c0 + 512], rp[:])
                hlo, hhi = half * 1024, (half + 1) * 1024
                for widx, (w0, w1) in enumerate(allwins):
                    src_end = w0 + 128
                    if not (hlo < src_end <= hhi):
                        continue
                    tp = AP_.tile([128, 128], f32r, tag="st", name="tp")
                    nc.tensor.transpose(tp[:], vt_sb[:, w0:w0 + 128], id_sb)
                    dst = vga[:, widx, :].rearrange(
                        "p (a c) -> p a c", a=2, c=65)[:, :, 0:64]
                    s2 = tp[:].rearrange("p (a c) -> p a c", a=2, c=64)
                    if widx % 2:
                        nc.scalar.copy(dst, s2)
                    else:
                        nc.vector.tensor_copy(dst, s2)

            def attention_chunk(h, lo, hi, clo, chi):
                    kts = winidx[(lo, hi)]
                    cw = chi - clo
                    # fp32r matmuls need an even moving-dim size; pad the
                    # compute window (extra column is masked -> zeros) and
                    # slice the copies to the true range.
                    plo, phi = clo, chi
                    if cw % 2:
                        if phi < S:
                            phi += 1
                        else:
                            plo -= 1
                    cwp = phi - plo
                    off = clo - plo
                    pot = BP.tile([65, 512], f32, tag="po", name="pot")

                    def _av(ti, win, u):
                        widx, w0, w1 = win
                        wd = w1 - w0
                        nc.tensor.matmul(
                            pot[:, 0:cwp], vg[h][0:wd, widx, :],
                            u[0:wd, 0:cwp],
                            start=(ti == 0), stop=(ti == len(kts) - 1))

                    pend = None
                    for ti, win in enumerate(kts):
                        widx, w0, w1 = win
                        wd = w1 - w0
                        stl = AP_.tile([128, 512], f32, tag="st", name="stl")
                        nc.tensor.matmul(stl[0:wd, 0:cwp],
                                         ka[h][0:69, w0:w1],
                                         qa[h][0:69, plo:phi],
                                         start=True, stop=True)
                        if pend is not None:
                            _av(*pend)
                        u = upool.tile([128, 512], f32r, tag="u", name="u")
                        nc.scalar.activation(u[0:wd, 0:cwp],
                                             stl[0:wd, 0:cwp], AF.Exp)
                        pend = (ti, win, u)
                    _av(*pend)
                    # r for these columns is complete; renormalize while
                    # draining O out of psum.  1/r is broadcast across the 64
                    # partitions with a partition-step-0 DMA.
                    nc.vector.tensor_copy(rsb[h][64:65, clo:chi],
                                          pot[64:65, off:off + cw])
                    if cwp != cw:
                        pad = chi if phi > chi else plo
                        nc.gpsimd.memset(
                            rsb[h][64:65, pad:pad + 1].bitcast(mybir.dt.uint32),
                            0x3F800000)
                    rt = FP.tile([64, 512], f32, tag="rp", name="rt")
                    nc.tensor.matmul(rt[:, 0:cwp], ones_sb[64:65, :],
                                     rsb[h][64:65, plo:phi],
                                     start=True, stop=True)
                    rv = ocp.tile([64, 512], f32, tag="rv", name="rv")
                    nc.vector.reciprocal_approx_fast(rv[:, 0:cwp], rt[:, 0:cwp])
                    nc.vector.tensor_mul(osb[h][0:64, clo:chi],
                                         pot[0:64, off:off + cw],
                                         rv[:, off:off + cw])

            def tail_chunk(j):
                for i in range(4 * j, 4 * j + 4):
                    oc = ocp.tile([128, 1024], f32, tag="oc", name="oc")
                    for jj in (0, 1):
                        pout = FP.tile([128, 512], f32,
                                       tag=f"pqk{npout[0] % 2}", name="pout")
                        npout[0] += 1
                        nc.tensor.matmul(pout[:], osb[0][:, ts(i, 128)],
                                         wo0_sb[:, ts(jj, 512)],
                                         start=True, stop=False)
                        nc.tensor.matmul(pout[:], osb[1][:, ts(i, 128)],
                                         wo1_sb[:, ts(jj, 512)],
                                         start=False, stop=True)
                        if (i + jj) % 2:
                            nc.scalar.copy(oc[:, ts(jj, 512)], pout[:])
                        else:
                            nc.vector.tensor_copy(oc[:, ts(jj, 512)], pout[:])
                    nc.sync.dma_start(out_d[ts(i, 128), :], oc[:])

            front_half(0)
            front_half(1)
            done = 0
            for si, (lo, hi) in enumerate(segs):
                for (clo, chi) in _chunks(lo, hi):
                    for h in (0, 1):
                        attention_chunk(h, lo, hi, clo, chi)
                last = si == len(segs) - 1
                while (done + 1) * 512 <= hi or (last and done < 4):
                    tail_chunk(done)
                    done += 1

    nc.compile()
    return nc


def _host_tensors(x, seg, fc, fs, wq, wk, wv, wo):
    c64 = np.repeat(fc.T, 2, axis=0)
    s64 = np.empty((64, S), np.float32)
    s64[0::2] = fs.T
    s64[1::2] = -fs.T
    cos2 = np.ascontiguousarray(np.tile(c64, (2, 1)))
    sin2 = np.ascontiguousarray(np.tile(s64, (2, 1)))

    A = (seg[None, :] == np.arange(4)[:, None]).astype(np.float32)
    maskq = np.ascontiguousarray(np.concatenate([np.ones((1, S), np.float32), A]))
    maskk = np.ascontiguousarray(
        np.concatenate([np.full((1, S), -256.0, np.float32), 256.0 * A]))

    sel = np.zeros((4, 128, 64), np.float32)
    for h in (0, 1):
        for j in range(64):
            sel[h, h * 64 + (j ^ 1), j] = 1.0
            sel[2 + h, h * 64 + j, j] = 1.0
    sels = np.ascontiguousarray(sel.transpose(1, 0, 2)).reshape(128, 256)

    idon = np.zeros((128, 192), np.float32)
    idon[:, 0:128] = np.eye(128, dtype=np.float32)
    idon[:, 128:192] = 1.0

    xts = np.ascontiguousarray(
        x.T.reshape(8, 128, S).transpose(1, 0, 2)).reshape(128, 8 * S)

    def wstack(w, scale):
        out = []
        for m in range(NCORES):
            wl = (w[m * 128:(m + 1) * 128, :] * scale).T.astype(np.float32)
            out.append(np.ascontiguousarray(
                wl.reshape(8, 128, 128).transpose(1, 0, 2)).reshape(128, 1024))
        return out

    wqs = wstack(wq, 1.0 / 8.0)
    wks = wstack(wk, 1.0)
    wvs = wstack(wv, 1.0)
    wos = [np.ascontiguousarray(wo[:, m * 128:(m + 1) * 128].T)
           for m in range(NCORES)]

    common = {"xts": xts, "cs2": cos2, "sn2": sin2, "mskq": maskq,
              "mskk": maskk, "sels": sels, "idon": idon}
    in_maps = []
    for m in range(NCORES):
        im = dict(common)
        im["wqs"] = wqs[m]
        im["wks"] = wks[m]
        im["wvs"] = wvs[m]
        im["wos"] = wos[m]
        in_maps.append(im)
    return in_maps


def kernel(x, seg_ids, freqs_cos, freqs_sin, wq, wk, wv, wo):
    x = np.asarray(x, np.float32).reshape(S, D)
    seg = np.asarray(seg_ids).astype(np.int64)
    fc = np.asarray(freqs_cos, np.float32)
    fs = np.asarray(freqs_sin, np.float32)
    wq = np.asarray(wq, np.float32)
    wk = np.asarray(wk, np.float32)
    wv = np.asarray(wv, np.float32)
    wo = np.asarray(wo, np.float32)

    bounds = tuple(int(b) for b in np.searchsorted(seg, np.arange(5)))
    if bounds not in _PROG_CACHE:
        _PROG_CACHE[bounds] = _build(bounds)
    nc = _PROG_CACHE[bounds]

    in_maps = _host_tensors(x, seg, fc, fs, wq, wk, wv, wo)

    from concourse.bass_utils import run_bass_kernel_spmd

    trace = bool(os.environ.get("BASS_KERNEL_TRACE"))
    res = run_bass_kernel_spmd(nc, in_maps, core_ids=list(range(NCORES)),
                               trace=trace)
    if trace and res.exec_time_ns is not None:
        print(f"HW exec time: {res.exec_time_ns} ns")
        if res.instructions_and_trace is not None:
            print("trace:", res.instructions_and_trace[1])

    out = np.sum(np.stack([r["outp"] for r in res.results]), axis=0,
                 dtype=np.float64)
    return out.astype(np.float32).reshape(1, S, D)



# revision 33
# speedup vs baseline: 1.5769x; 1.5769x over previous
"""Trainium2 Bass kernel for nn_Attention_11458972746115.

Multi-head attention (B=1, S=2048, D=1024, H=16, DH=64) with RoPE and a
block-diagonal segment mask, tensor-parallel over heads across 8 NeuronCores
(2 heads per core).  Each core computes qkv projections, RoPE, block-sparse
masked attention and its slice of the output projection; the partial output
products (sum-sharded over the wo contraction) are reduced on the host.

v3 design (vs the f32r baseline):
 - All matmuls run in bf16 (1 cycle/row, like f32r, but half the SBUF/HBM
   traffic).  fp8 was evaluated and rejected: per-element quantization error
   does not average down over the contraction, giving ~7% output error vs
   the 2e-2 gate.
 - The block-diagonal mask costs nothing: k-windows are clipped per segment
   (partial-width transposes/stationaries), so no mask rows are needed; the
   1/sqrt(DH) score scale rides the exp activation's scale immediate.
 - RoPE: q_rot = q*cos + P@(q*sin') with P the pair-swap permutation; one
   128-wide PE matmul per projection-chunk covers both heads, and the
   identity half of the combine is folded into the drain add on DVE/Pool.
 - Softmax renormalization rides the attn@v matmul via an appended ones
   column in v (head0 at psum row 64; head1's v block sits at psum rows
   64..127 with its ones column at row 32), so both heads' attention
   outputs land partition-aligned in one [128,S] osb tile and the output
   projection contracts over all 128 dims in single matmuls.
 - HBM traffic is bf16 both ways; out partials are summed on the host.
"""

import os
import numpy as np

S = 2048
D = 1024
H = 16
DH = 64
NCORES = 8

_PROG_CACHE = {}


def _chunks(lo, hi, maxw=512):
    n = -(-(hi - lo) // maxw)
    base = (hi - lo) // n
    rem = (hi - lo) % n
    out = []
    p = lo
    for i in range(n):
        w = base + (1 if i < rem else 0)
        out.append((p, p + w))
        p += w
    return out


def _build(bounds, reps=1):
    import contextlib

    import concourse.bacc as bacc
    import concourse.mybir as mybir
    import concourse.tile as tile
    from concourse.bass import ts

    f32 = mybir.dt.float32
    f32r = mybir.dt.float32r
    bf16 = mybir.dt.bfloat16
    ew = bf16 if os.environ.get("KBF16") else f32r
    AF = mybir.ActivationFunctionType

    segs = [(bounds[g], bounds[g + 1]) for g in range(4) if bounds[g + 1] > bounds[g]]

    allwins = []
    winidx = {}
    for (lo, hi) in segs:
        lst = []
        for w0 in range(lo, hi, 128):
            w1 = min(w0 + 128, hi)
            lst.append((len(allwins), w0, w1))
            allwins.append((w0, w1))
        winidx[(lo, hi)] = lst
    NW = len(allwins)

    nc = bacc.Bacc(None, target_bir_lowering=False)

    xq_d = nc.dram_tensor("xq3", [128, 8, S], bf16, kind="ExternalInput")
    wq_d = nc.dram_tensor("wq3", [128, 8, 128], bf16, kind="ExternalInput")
    wk_d = nc.dram_tensor("wk3", [128, 8, 128], bf16, kind="ExternalInput")
    wv_d = nc.dram_tensor("wv3", [128, 8, 128], bf16, kind="ExternalInput")
    wo_d = nc.dram_tensor("wo3", [128, 1024], mybir.dt.float32r if not os.environ.get("KBF16") else bf16, kind="ExternalInput")
    cs_d = nc.dram_tensor("cs2", [128, S], bf16, kind="ExternalInput")
    sn_d = nc.dram_tensor("sn2", [128, S], bf16, kind="ExternalInput")
    ax_d = nc.dram_tensor("aux2", [128, 256], mybir.dt.float32r if not os.environ.get("KBF16") else bf16, kind="ExternalInput")
    out_d = nc.dram_tensor("outp", [S, D], bf16 if os.environ.get("KBF16") else f32, kind="ExternalOutput")
    dbg = bool(os.environ.get("KDBG"))
    if dbg:
        ewd = mybir.dt.bfloat16 if os.environ.get("KBF16") else mybir.dt.float32r
        osb_dump = nc.dram_tensor("dbg_osb", [128, S], ewd, kind="ExternalOutput")
        qa_dump = nc.dram_tensor("dbg_qa", [128, S], ewd, kind="ExternalOutput")
        ka_dump = nc.dram_tensor("dbg_ka", [128, S], ewd, kind="ExternalOutput")
        vt_dump = nc.dram_tensor("dbg_vt", [128, S], ewd, kind="ExternalOutput")
        rsb_dump = nc.dram_tensor("dbg_rsb", [65, S], ewd, kind="ExternalOutput")
        aq_dump = nc.dram_tensor("dbg_aq3", [128, 512], ewd, kind="ExternalOutput")
        bq_dump = nc.dram_tensor("dbg_bq3", [128, 512], ewd, kind="ExternalOutput")
        rpq_dump = nc.dram_tensor("dbg_rpq3", [128, 512], mybir.dt.float32, kind="ExternalOutput")
        qb_dump = nc.dram_tensor("dbg_qb3", [128, 512], mybir.dt.float32, kind="ExternalOutput")
        xc_dump = nc.dram_tensor("dbg_xc3", [128, 8, 512], mybir.dt.bfloat16, kind="ExternalOutput")
        wq_dump = nc.dram_tensor("dbg_wq", [128, 8, 128], mybir.dt.bfloat16, kind="ExternalOutput")
        wo_dump = nc.dram_tensor("dbg_wo", [128, 1024], mybir.dt.bfloat16 if os.environ.get("KBF16") else mybir.dt.float32r, kind="ExternalOutput")
        po_dump = nc.dram_tensor("dbg_po", [128, 1024], mybir.dt.float32, kind="ExternalOutput")
        cs_dump = nc.dram_tensor("dbg_cs", [128, S], mybir.dt.bfloat16, kind="ExternalOutput")
        sn_dump = nc.dram_tensor("dbg_sn", [128, S], mybir.dt.bfloat16, kind="ExternalOutput")
        pot_dump = nc.dram_tensor("dbg_pot1", [128, 512], mybir.dt.float32, kind="ExternalOutput")
        rv_dump = nc.dram_tensor("dbg_rv1", [128, 512], mybir.dt.float32, kind="ExternalOutput")
        rt_dump = nc.dram_tensor("dbg_rt1", [128, 512], mybir.dt.float32, kind="ExternalOutput")

    with tile.TileContext(nc) as tc:
        rep_ctx = (tc.For_i(0, reps, 1, hint_engines=(
            mybir.EngineType.PE, mybir.EngineType.DVE,
            mybir.EngineType.Activation, mybir.EngineType.SP,
            mybir.EngineType.Pool))
                   if reps > 1 else contextlib.nullcontext())
        with rep_ctx, \
             tc.tile_pool(name="fp", bufs=1, space="PSUM") as FP, \
             tc.tile_pool(name="apsum", bufs=3, space="PSUM") as AP_, \
             tc.tile_pool(name="bp", bufs=2, space="PSUM") as BP, \
             tc.tile_pool(name="const", bufs=1) as const, \
             tc.tile_pool(name="pers", bufs=1) as pers, \
             tc.tile_pool(name="abp", bufs=2) as abp, \
             tc.tile_pool(name="up", bufs=3) as upool, \
             tc.tile_pool(name="ocp", bufs=3) as ocp:

            wq_sb = const.tile([128, 8, 128], bf16)
            wk_sb = const.tile([128, 8, 128], bf16)
            wv_sb = const.tile([128, 8, 128], bf16)
            wo_sb = const.tile([128, 1024], wo_d.dtype)
            cs_sb = const.tile([128, S], bf16)
            sn_sb = const.tile([128, S], bf16)
            ax_sb = const.tile([128, 256], ax_d.dtype)
            o1_sb = const.tile([128, 128], ew)

            xq_sb = pers.tile([128, 8, S], bf16)
            qa = pers.tile([128, S], ew, name="qa")
            ka = pers.tile([128, S], ew, name="ka")
            vt_sb = pers.tile([128, S], ew, name="vt")
            vg0 = pers.tile([128, NW, 65], ew, name="vg0")
            vg1 = pers.tile([128, NW, 128], ew, name="vg1")
            rsb = pers.tile([65, S], ew, name="rsb")
            osb = pers.tile([128, S], ew, name="osb")

            # input DMAs, ordered so the first projection matmuls (wq + x
            # chunk 0) are gated by as little traffic as possible
            nc.sync.dma_start(wq_sb[:], wq_d[:])
            nc.sync.dma_start(xq_sb[:, 0:4, ts(0, 512)], xq_d[:, 0:4, ts(0, 512)])
            nc.sync.dma_start(xq_sb[:, 4:8, ts(0, 512)], xq_d[:, 4:8, ts(0, 512)])
            nc.sync.dma_start(wk_sb[:], wk_d[:])
            nc.sync.dma_start(cs_sb[:], cs_d[:])
            nc.sync.dma_start(sn_sb[:], sn_d[:])
            nc.sync.dma_start(wv_sb[:], wv_d[:])
            nc.sync.dma_start(xq_sb[:, :, ts(1, 512)], xq_d[:, :, ts(1, 512)])
            nc.sync.dma_start(ax_sb[:], ax_d[:])
            nc.sync.dma_start(xq_sb[:, :, ts(2, 512)], xq_d[:, :, ts(2, 512)])
            nc.sync.dma_start(xq_sb[:, :, ts(3, 512)], xq_d[:, :, ts(3, 512)])
            nc.sync.dma_start(wo_sb[:], wo_d[:])

            P_sb = ax_sb[:, 0:128]
            id_sb = ax_sb[:, 128:256]
            if ew == bf16:
                one_bits, one_dt = 0x3F80, mybir.dt.uint16
            else:
                one_bits, one_dt = 0x3F800000, mybir.dt.uint32
            # rank-1 softmax-denominator broadcast constant (1.0)
            nc.gpsimd.memset(o1_sb[:].bitcast(one_dt), one_bits)
            # ones columns: head0's at vg0 col 64, head1's at vg1 col 0
            nc.gpsimd.memset(vg0[:, :, 64:65].bitcast(one_dt), one_bits)
            nc.gpsimd.memset(vg1[:, :, 0:1].bitcast(one_dt), one_bits)
            nc.gpsimd.memset(vg1[:, :, 1:64].bitcast(one_dt), 0)
            if dbg:
                nc.gpsimd.memset(rsb[:].bitcast(one_dt), 0)
                nc.gpsimd.memset(osb[:].bitcast(one_dt), 0)
                nc.gpsimd.memset(qa[:].bitcast(one_dt), 0)
                nc.gpsimd.memset(ka[:].bitcast(one_dt), 0)
                nc.gpsimd.memset(vt_sb[:].bitcast(one_dt), 0)

            ntag = [0]
            wdone = [0]

            def front_chunk(c):
                sl = ts(c, 512)
                qb = FP.tile([128, 512], f32, tag=f"qkv{ntag[0] % 2}", name=f"qb{c}")
                ntag[0] += 1
                kb = FP.tile([128, 512], f32, tag=f"qkv{ntag[0] % 2}", name=f"kb{c}")
                ntag[0] += 1
                for j in range(8):
                    nc.tensor.matmul(qb[:], wq_sb[:, j, :], xq_sb[:, j, sl],
                                     start=(j == 0), stop=(j == 7))
                for j in range(8):
                    nc.tensor.matmul(kb[:], wk_sb[:, j, :], xq_sb[:, j, sl],
                                     start=(j == 0), stop=(j == 7))
                aq = abp.tile([128, 512], ew, tag="aq", name=f"aq{c}")
                bq = abp.tile([128, 512], ew, tag="bq", name=f"bq{c}")
                ak = abp.tile([128, 512], ew, tag="ak", name=f"ak{c}")
                bk = abp.tile([128, 512], ew, tag="bk", name=f"bk{c}")
                nc.vector.tensor_mul(aq[:], qb[:], cs_sb[:, sl])
                nc.vector.tensor_mul(bq[:], qb[:], sn_sb[:, sl])
                nc.vector.tensor_mul(ak[:], kb[:], cs_sb[:, sl])
                nc.vector.tensor_mul(bk[:], kb[:], sn_sb[:, sl])
                rpq = FP.tile([128, 512], f32, tag="rp", name=f"rpq{c}")
                nc.tensor.matmul(rpq[:], P_sb, bq[:], start=True, stop=True)
                nc.vector.tensor_add(qa[:, sl], rpq[:], aq[:])
                if dbg and c == 3:
                    rpq_st = pers.tile([128, 512], f32, name="rpq_st")
                    nc.scalar.copy(rpq_st[:], rpq[:])
                    qb_st = pers.tile([128, 512], f32, name="qb_st")
                    nc.scalar.copy(qb_st[:], qb[:])
                    nc.sync.dma_start(qb_dump[:], qb_st[:])
                    nc.sync.dma_start(xc_dump[:], xq_sb[:, :, sl])
                    nc.sync.dma_start(aq_dump[:], aq[:])
                    nc.sync.dma_start(bq_dump[:], bq[:])
                    nc.sync.dma_start(rpq_dump[:], rpq_st[:])
                vb = FP.tile([128, 512], f32, tag=f"qkv{ntag[0] % 2}", name=f"vb{c}")
                ntag[0] += 1
                for j in range(8):
                    nc.tensor.matmul(vb[:], wv_sb[:, j, :], xq_sb[:, j, sl],
                                     start=(j == 0), stop=(j == 7))
                rpk = FP.tile([128, 512], f32, tag="rp", name=f"rpk{c}")
                nc.tensor.matmul(rpk[:], P_sb, bk[:], start=True, stop=True)
                nc.vector.tensor_add(ka[:, sl], rpk[:], ak[:])
                nc.scalar.copy(vt_sb[:, sl], vb[:])
                # transposes for windows now fully resident
                loaded = 512 * (c + 1)
                while wdone[0] < NW and allwins[wdone[0]][1] <= loaded:
                    widx = wdone[0]
                    w0, w1 = allwins[widx]
                    wd = w1 - w0
                    wde = min(wd + (wd & 1), S - w0)
                    tp = AP_.tile([128, 128], f32, tag="st", name=f"tp{widx}")
                    nc.tensor.matmul(tp[0:wde, :], vt_sb[:, w0:w0 + wde], id_sb,
                                     start=True, stop=True)
                    nc.scalar.copy(vg0[0:wde, widx, 0:64], tp[0:wde, 0:64])
                    nc.vector.tensor_copy(vg1[0:wde, widx, 64:128],
                                          tp[0:wde, 64:128])
                    wdone[0] += 1

            nosb = [0]

            def attention_chunk(h, lo, hi, clo, chi):
                kts = winidx[(lo, hi)]
                cw = chi - clo
                # f32r broadcast matmul needs an even moving size; pad the
                # compute window (the extra column computes garbage that is
                # simply never copied out).
                plo, phi = clo, chi
                if cw % 2:
                    if phi < S:
                        phi += 1
                    else:
                        plo -= 1
                cwp = phi - plo
                off = clo - plo
                hs = 64 * h
                rrow = 64 if h == 0 else 0
                pot = BP.tile([128, 512], f32, tag="po", name=f"pot{h}")

                def _av(ti, win, u):
                    widx, w0, w1 = win
                    wd = w1 - w0
                    if h == 0:
                        nc.tensor.matmul(pot[0:65, 0:cwp], vg0[0:wd, widx, :],
                                         u[0:wd, 0:cwp],
                                         start=(ti == 0), stop=(ti == len(kts) - 1))
                    else:
                        nc.tensor.matmul(pot[:, 0:cwp], vg1[0:wd, widx, :],
                                         u[0:wd, 0:cwp],
                                         start=(ti == 0), stop=(ti == len(kts) - 1))

                pend = None
                for ti, win in enumerate(kts):
                    widx, w0, w1 = win
                    wd = w1 - w0
                    stl = AP_.tile([128, 512], f32, tag="st", name="stl")
                    nc.tensor.matmul(stl[0:wd, 0:cwp],
                                     ka[hs:hs + 64, w0:w1],
                                     qa[hs:hs + 64, plo:phi],
                                     start=True, stop=True)
                    if pend is not None:
                        _av(*pend)
                    u = upool.tile([128, 512], ew, tag="u", name="u")
                    nc.scalar.activation(u[0:wd, 0:cwp], stl[0:wd, 0:cwp],
                                         AF.Exp, scale=0.125)
                    pend = (ti, win, u)
                _av(*pend)
                # r to sbuf (f32r), broadcast over the head's 64 partitions
                # with a rank-1 matmul, then reciprocal + renormalize.
                nc.vector.tensor_copy(rsb[rrow:rrow + 1, plo:phi],
                                      pot[rrow:rrow + 1, 0:cwp])
                rt = FP.tile([128, 512], f32, tag="rp", name="rt")
                nc.tensor.matmul(rt[:, 0:cwp], o1_sb[rrow:rrow + 1, :],
                                 rsb[rrow:rrow + 1, plo:phi],
                                 start=True, stop=True)
                rv = upool.tile([128, 512], f32, tag="rv", name="rv")
                if dbg:
                    nc.gpsimd.memset(rv[:].bitcast(mybir.dt.uint32), 0)
                # NOTE: reciprocal_approx_fast is a custom DVE ucode op that
                # silently writes zeros when invoked at partition base != 0;
                # always run it over the full 128 partitions (rt is a
                # broadcast, so all rows hold valid data).
                nc.vector.reciprocal_approx_fast(rv[:, 0:cwp], rt[:, 0:cwp])
                nc.vector.tensor_mul(osb[hs:hs + 64, clo:chi],
                                     pot[hs:hs + 64, off:off + cw],
                                     rv[hs:hs + 64, off:off + cw])
                if dbg and h == 1 and clo == segs[0][0]:
                    pot_st = pers.tile([128, 512], f32, name="pot_st")
                    rt_st = pers.tile([128, 512], f32, name="rt_st")
                    nc.scalar.copy(pot_st[:, 0:cwp], pot[:, 0:cwp])
                    nc.scalar.copy(rt_st[:, 0:cwp], rt[:, 0:cwp])
                    nc.gpsimd.memset(pot_st[:, cwp:512].bitcast(mybir.dt.uint32), 0)
                    nc.gpsimd.memset(rt_st[:, cwp:512].bitcast(mybir.dt.uint32), 0)
                    nc.sync.dma_start(pot_dump[:], pot_st[:])
                    nc.sync.dma_start(rt_dump[:], rt_st[:])
                    nc.sync.dma_start(rv_dump[:], rv[:])
                nosb[0] += 1

            def tail_chunk(jt):
                for i in range(4 * jt, 4 * jt + 4):
                    oc = ocp.tile([128, 1024], bf16 if os.environ.get("KBF16") else f32, tag="oc", name="oc")
                    for jj in (0, 1):
                        pout = AP_.tile([128, 512], f32, tag="st", name="pout")
                        nc.tensor.matmul(pout[:], osb[:, ts(i, 128)],
                                         wo_sb[:, ts(jj, 512)],
                                         start=True, stop=True)
                        if (i * 2 + jj) % 2 == 0:
                            nc.vector.tensor_copy(oc[:, ts(jj, 512)], pout[:])
                        else:
                            nc.scalar.copy(oc[:, ts(jj, 512)], pout[:])
                        if dbg and i == 0:
                            po_st = pers.tile([128, 1024], f32, name=f"po_st{jj}")
                            nc.vector.tensor_copy(po_st[:, 0:512], pout[:])
                            nc.sync.dma_start(po_dump[:, ts(jj, 512)], po_st[:, 0:512])
                    if i % 2:
                        nc.scalar.dma_start(out_d[ts(i, 128), :], oc[:])
                    else:
                        nc.sync.dma_start(out_d[ts(i, 128), :], oc[:])

            # Emission: interleave projection chunks with attention on the
            # segments that become ready, so Act/DVE work overlaps PE.
            seg_after_chunk = {}
            for si, (lo, hi) in enumerate(segs):
                cmin = -(-hi // 512) - 1
                seg_after_chunk.setdefault(cmin, []).append(si)

            done = 0
            if os.environ.get("KSEQ"):
                seg_after_chunk = {3: list(range(len(segs)))}
            for c in range(4):  # noqa
                front_chunk(c)
                for si in seg_after_chunk.get(c, []):
                    lo, hi = segs[si]
                    for (clo, chi) in _chunks(lo, hi):
                        for h in (0, 1):
                            attention_chunk(h, lo, hi, clo, chi)
                    last = si == len(segs) - 1
                    while (done + 1) * 512 <= hi or (last and done < 4):
                        tail_chunk(done)
                        done += 1
            if dbg:
                nc.sync.dma_start(osb_dump[:], osb[:])
                nc.sync.dma_start(qa_dump[:], qa[:])
                nc.sync.dma_start(ka_dump[:], ka[:])
                nc.sync.dma_start(vt_dump[:], vt_sb[:])
                nc.sync.dma_start(rsb_dump[:], rsb[:])
                nc.sync.dma_start(wq_dump[:], wq_sb[:])
                nc.sync.dma_start(wo_dump[:], wo_sb[:])
                nc.sync.dma_start(cs_dump[:], cs_sb[:])
                nc.sync.dma_start(sn_dump[:], sn_sb[:])

    nc.compile()
    return nc


def _host_tensors(x, seg, fc, fs, wq, wk, wv, wo):
    import ml_dtypes

    bf16 = ml_dtypes.bfloat16

    # cos/sin tables: pair-repeated cos, sign-alternating sin, tiled to 128
    # partitions (the two heads handled per core share the pattern).
    c64 = np.repeat(fc.T, 2, axis=0)
    s64 = np.empty((64, S), np.float32)
    s64[0::2] = fs.T
    s64[1::2] = -fs.T
    cos2 = np.tile(c64, (2, 1)).astype(bf16)
    sin2 = np.tile(s64, (2, 1)).astype(bf16)

    # aux: pair-swap permutation P and identity (for transposes)
    aux = np.zeros((128, 256), np.float32)
    for j in range(128):
        aux[j ^ 1, j] = 1.0          # P
        aux[j, 128 + j] = 1.0        # I
    ewnp = bf16 if os.environ.get("KBF16") else np.float32
    aux = np.ascontiguousarray(aux).astype(ewnp)

    xq3 = np.ascontiguousarray(
        x.T.reshape(8, 128, S).transpose(1, 0, 2)).astype(bf16)

    def wstack(w):
        out = []
        for m in range(NCORES):
            wl = w[m * 128:(m + 1) * 128, :].T.astype(np.float32)
            out.append(np.ascontiguousarray(
                wl.reshape(8, 128, 128).transpose(1, 0, 2)).astype(bf16))
        return out

    wqs = wstack(wq)
    wks = wstack(wk)
    wvs = wstack(wv)
    wos = [np.ascontiguousarray(wo[:, m * 128:(m + 1) * 128].T).astype(ewnp)
           for m in range(NCORES)]

    common = {"xq3": xq3, "cs2": cos2, "sn2": sin2, "aux2": aux}
    in_maps = []
    for m in range(NCORES):
        im = dict(common)
        im["wq3"] = wqs[m]
        im["wk3"] = wks[m]
        im["wv3"] = wvs[m]
        im["wo3"] = wos[m]
        in_maps.append(im)
    return in_maps


def kernel(x, seg_ids, freqs_cos, freqs_sin, wq, wk, wv, wo):
    x = np.asarray(x, np.float32).reshape(S, D)
    seg = np.asarray(seg_ids).astype(np.int64)
    fc = np.asarray(freqs_cos, np.float32)
    fs = np.asarray(freqs_sin, np.float32)
    wq = np.asarray(wq, np.float32)
    wk = np.asarray(wk, np.float32)
    wv = np.asarray(wv, np.float32)
    wo = np.asarray(wo, np.float32)

    bounds = tuple(int(b) for b in np.searchsorted(seg, np.arange(5)))
    if bounds not in _PROG_CACHE:
        _PROG_CACHE[bounds] = _build(bounds)
    nc = _PROG_CACHE[bounds]

    in_maps = _host_tensors(x, seg, fc, fs, wq, wk, wv, wo)

    from concourse.bass_utils import run_bass_kernel_spmd

    trace = bool(os.environ.get("BASS_KERNEL_TRACE"))
    res = run_bass_kernel_spmd(nc, in_maps, core_ids=list(range(NCORES)),
                               trace=trace)
    if trace and res.exec_time_ns is not None:
        print(f"HW exec time: {res.exec_time_ns} ns")

    out = np.sum(np.stack([np.asarray(r["outp"], np.float32)
                           for r in res.results]), axis=0)
    return out.astype(np.float32).reshape(1, S, D)
